# revision 1
# baseline (speedup 1.0000x reference)
"""MoE-LoRA Linear kernel for 8 Trainium2 NeuronCores.

Sharding: core c -> (batch b = c//2, out-feature half = c%2).
Each core computes out[b, :, half] = x[b] @ W_half.T + b_half
                                   + SCALING * router-weighted LoRA.
Router (softmax over mean-pooled x[b]) is computed redundantly per core —
it only needs that core's batch, so there are no collectives.

Device layout (per core):
  xT  [4096, 2048] f32r   x[b].T (d-major), processed in two t-panels of 1024
  Wt  [4096, 2048] f32r   W_base[half].T  (streamed per panel)
  At  [4096, 64]   f32r   lora_A as [d, e*8+r]
  Bta [65, 2048]   f32r   rows 0-63: lora_B[half] as [er, o]; row 64: b_base
  rW  [4096, 8]    f32r   router_W.T
  out [2048, 2048] f32    result transposed: [o, t]

Main matmul: psum[o128, t512] += Wt_tile[d128, o128].T @ xT[d128, t512]
over 32 d-tiles, then one K=65 matmul adds router-weighted LoRA + bias
(row 64 of Bta is the bias, matched by a ones-row in the augmented h).
"""
import sys

sys.path.insert(0, "/opt/trn_rl_repo")

import numpy as np

import concourse.bass as bass
import concourse.mybir as mybir
import concourse.tile as tile
from concourse import bacc, bass_isa
from concourse.bass_utils import run_bass_kernel_spmd

F32 = mybir.dt.float32
F32R = mybir.dt.float32r

D, T, O_SH, E, R = 4096, 2048, 2048, 8, 8
ER = E * R  # 64
DT = D // 128  # 32 d-tiles
TP = 1024  # t-panel size
N_PANEL = T // TP  # 2
OT = O_SH // 128  # 16 o-tiles
ROUTER_TEMP = 1.0
SCALING = 16.0 / 8.0

_nc_cache = []


def build():
    nc = bacc.Bacc(None, target_bir_lowering=False)
    xT = nc.dram_tensor("xT", [D, T], F32R, kind="ExternalInput")
    Wt = nc.dram_tensor("Wt", [D, O_SH], F32R, kind="ExternalInput")
    At = nc.dram_tensor("At", [D, ER], F32R, kind="ExternalInput")
    Bta = nc.dram_tensor("Bta", [ER + 1, O_SH], F32R, kind="ExternalInput")
    rW = nc.dram_tensor("rW", [D, E], F32R, kind="ExternalInput")
    rb = nc.dram_tensor("rb", [E], F32, kind="ExternalInput")
    ones_d = nc.dram_tensor("ones_d", [T], F32R, kind="ExternalInput")
    out = nc.dram_tensor("out", [O_SH, T], F32, kind="ExternalOutput")
    wscratch = nc.dram_tensor("wscratch", [E], F32)

    with tile.TileContext(nc) as tc:
        with (
            tc.tile_pool(name="xpool", bufs=1) as xpool,
            tc.tile_pool(name="wpool", bufs=2) as wpool,
            tc.tile_pool(name="single", bufs=1) as single,
            tc.tile_pool(name="ev", bufs=2) as evpool,
            tc.tile_pool(name="ps", bufs=2, space="PSUM") as psp,
            tc.tile_pool(name="psmain", bufs=4, space="PSUM") as psm,
            tc.tile_pool(name="ps3", bufs=2, space="PSUM") as ps3,
        ):
            atp = single.tile([128, DT, ER], F32R)
            nc.sync.dma_start(atp[:], At[:].rearrange("(dt p) r -> p dt r", p=128))
            rwp = single.tile([128, DT, E], F32R)
            nc.sync.dma_start(rwp[:], rW[:].rearrange("(dt p) e -> p dt e", p=128))
            bta = single.tile([ER + 1, O_SH], F32R)
            nc.sync.dma_start(bta[:], Bta[:])
            rb8 = single.tile([E, 1], F32)
            nc.sync.dma_start(rb8[:], rb[:, None])
            haug = single.tile([ER + 1, T], F32R)
            nc.sync.dma_start(haug[ER : ER + 1, :], ones_d[None, :])

            # xsum partials: [128, DT, 4] per quarter-panel slice; 2 panels
            xq = single.tile([128, DT, 4 * N_PANEL], F32)
            xsum_r = single.tile([128, DT], F32R)

            xpA = xpool.tile([128, DT, 512], F32R, tag="xpA")
            xpB = xpool.tile([128, DT, 512], F32R, tag="xpB")
            xhalves = (xpA, xpB)
            xTr = xT[:].rearrange("(dt p) t -> p dt t", p=128)

            for panel in range(N_PANEL):
                t0 = panel * TP
                # x panel load: two half-tiles, two sub-DMAs each
                for half in range(2):
                    for q in range(2):
                        nc.sync.dma_start(
                            xhalves[half][:, :, q * 256 : (q + 1) * 256],
                            xTr[
                                :, :,
                                t0 + half * 512 + q * 256 : t0 + half * 512 + (q + 1) * 256,
                            ],
                        )
                # h matmuls for this panel
                for tch in range(TP // 512):
                    hps = psp.tile([ER, 512], F32, tag="hps")
                    for d in range(DT):
                        nc.tensor.matmul(
                            hps[:],
                            atp[:, d, :],
                            xhalves[tch][:, d, :],
                            start=(d == 0),
                            stop=(d == DT - 1),
                        )
                    nc.vector.tensor_copy(
                        haug[0:ER, t0 + tch * 512 : t0 + (tch + 1) * 512], hps[:]
                    )
                # xsum partial reduces
                for q in range(4):
                    nc.vector.reduce_sum(
                        xq[:, :, panel * 4 + q],
                        xhalves[q // 2][:, :, (q % 2) * 256 : (q % 2 + 1) * 256].bitcast(F32),
                        axis=mybir.AxisListType.X,
                    )

                if panel == N_PANEL - 1:
                    # router: logits = xsum @ rW / T + rb, softmax, *SCALING
                    xsum_f = single.tile([128, DT], F32)
                    nc.vector.reduce_sum(
                        xsum_f[:], xq[:], axis=mybir.AxisListType.X
                    )
                    lgps = psp.tile([E, 1], F32, tag="hps")
                    for d in range(DT):
                        nc.tensor.matmul(
                            lgps[:],
                            rwp[:, d, :].bitcast(F32),
                            xsum_f[:, d : d + 1],
                            start=(d == 0),
                            stop=(d == DT - 1),
                        )
                    lg8 = single.tile([E, 1], F32)
                    nc.scalar.activation(
                        lg8[:], lgps[:], mybir.ActivationFunctionType.Copy,
                        scale=1.0 / (T * ROUTER_TEMP),
                    )
                    nc.vector.tensor_tensor(lg8[:], lg8[:], rb8[:], mybir.AluOpType.add)
                    m8 = single.tile([E, 1], F32)
                    nc.gpsimd.partition_all_reduce(
                        m8[:], lg8[:], channels=E, reduce_op=bass_isa.ReduceOp.max
                    )
                    e8 = single.tile([E, 1], F32)
                    nc.vector.tensor_tensor(e8[:], lg8[:], m8[:], mybir.AluOpType.subtract)
                    nc.scalar.activation(e8[:], e8[:], mybir.ActivationFunctionType.Exp)
                    s8 = single.tile([E, 1], F32)
                    nc.gpsimd.partition_all_reduce(
                        s8[:], e8[:], channels=E, reduce_op=bass_isa.ReduceOp.add
                    )
                    r8 = single.tile([E, 1], F32)
                    nc.vector.reciprocal(r8[:], s8[:])
                    w8 = single.tile([E, 1], F32)
                    nc.vector.tensor_tensor(w8[:], e8[:], r8[:], mybir.AluOpType.mult)
                    nc.vector.tensor_scalar_mul(w8[:], w8[:], SCALING)
                    nc.sync.dma_start(wscratch[:], w8[:, 0])
                    wexp = single.tile([ER + 1, 1], F32)
                    nc.vector.memset(wexp[ER : ER + 1, :], 1.0)
                    wsrc = bass.AP(tensor=wscratch, offset=0, ap=[[1, E], [0, R]])
                    nc.sync.dma_start(wexp[0:ER, :], wsrc)
                    # scale Bta rows by router weight (row 64 *= 1.0)
                    nc.vector.tensor_tensor(
                        bta[:], bta[:], wexp[:].to_broadcast([ER + 1, O_SH]),
                        mybir.AluOpType.mult,
                    )

                # main o-loop
                last = panel == N_PANEL - 1
                for o in range(OT):
                    osl = slice(o * 128, (o + 1) * 128)
                    wt = wpool.tile([128, DT, 128], F32R, tag="wt")
                    nc.sync.dma_start(
                        wt[:], Wt[:, osl].rearrange("(dt p) o -> p dt o", p=128)
                    )
                    pstiles = [
                        psm.tile([128, 512], F32, tag="main", name=f"main_{i}")
                        for i in range(2)
                    ]
                    for tch in range(2):
                        for d in range(DT):
                            nc.tensor.matmul(
                                pstiles[tch][:],
                                wt[:, d, :],
                                xhalves[tch][:, d, :],
                                start=(d == 0),
                                stop=(d == DT - 1) and not last,
                            )
                    if last:
                        for tch in range(2):
                            nc.tensor.matmul(
                                pstiles[tch][:],
                                bta[:, osl],
                                haug[:, t0 + tch * 512 : t0 + (tch + 1) * 512],
                                start=False,
                                stop=True,
                            )
                    for tch in range(2):
                        ev = evpool.tile([128, 512], F32, tag="ev")
                        nc.vector.tensor_copy(ev[:], pstiles[tch][:])
                        nc.sync.dma_start(
                            out[osl, t0 + tch * 512 : t0 + (tch + 1) * 512], ev[:]
                        )
                    if last:
                        # phase 3: lora+bias for panel 0 via DMA-accumulate
                        for tch in range(2):
                            p3 = ps3.tile([128, 512], F32, tag="p3")
                            nc.tensor.matmul(
                                p3[:],
                                bta[:, osl],
                                haug[:, tch * 512 : (tch + 1) * 512],
                                start=True,
                                stop=True,
                            )
                            ev3 = evpool.tile([128, 512], F32, tag="ev")
                            nc.vector.tensor_copy(ev3[:], p3[:])
                            nc.gpsimd.dma_start(
                                out[osl, tch * 512 : (tch + 1) * 512],
                                ev3[:],
                                accum_op=mybir.AluOpType.add,
                            )
    nc.compile()
    return nc


def _get_nc():
    if not _nc_cache:
        _nc_cache.append(build())
    return _nc_cache[0]


def kernel(x, W_base, b_base, lora_A, lora_B, router_W, router_b):
    x = np.asarray(x, dtype=np.float32)
    W_base = np.asarray(W_base, dtype=np.float32)
    b_base = np.asarray(b_base, dtype=np.float32)
    lora_A = np.asarray(lora_A, dtype=np.float32)
    lora_B = np.asarray(lora_B, dtype=np.float32)
    router_W = np.asarray(router_W, dtype=np.float32)
    router_b = np.asarray(router_b, dtype=np.float32)

    B, S, D_ = x.shape
    O = W_base.shape[0]
    At_h = np.ascontiguousarray(lora_A.reshape(E * R, D_).T)  # [D, 64]
    rW_h = np.ascontiguousarray(router_W.T)  # [D, 8]
    ones_h = np.ones(T, dtype=np.float32)

    in_maps = []
    for c in range(8):
        b, half = c // 2, c % 2
        osl = slice(half * O_SH, (half + 1) * O_SH)
        Bt = np.ascontiguousarray(
            lora_B[:, osl, :].transpose(0, 2, 1).reshape(E * R, O_SH)
        )
        Bta_h = np.concatenate([Bt, b_base[osl][None, :]], axis=0)
        in_maps.append(
            {
                "xT": np.ascontiguousarray(x[b].T),
                "Wt": np.ascontiguousarray(W_base[osl].T),
                "At": At_h,
                "Bta": np.ascontiguousarray(Bta_h),
                "rW": rW_h,
                "rb": router_b,
                "ones_d": ones_h,
            }
        )

    global _last_in_maps
    _last_in_maps = in_maps
    nc = _get_nc()
    res = run_bass_kernel_spmd(nc, in_maps, core_ids=list(range(8)))
    out = np.empty((B, S, O), dtype=np.float32)
    for c in range(8):
        b, half = c // 2, c % 2
        out[b, :, half * O_SH : (half + 1) * O_SH] = res.results[c]["out"].T
    return out



# revision 2
# speedup vs baseline: 1.1488x; 1.1488x over previous
"""MoE-LoRA Linear kernel for 8 Trainium2 NeuronCores.

Sharding: core c -> (batch b = c//2, out-feature half = c%2).
Each core computes out[b, :, half] = x[b] @ W_half.T + b_half
                                   + SCALING * router-weighted LoRA.

All matmul operands are bf16 (fp32 accumulation in PSUM); x is fully
resident in SBUF so W is streamed exactly once.

Device layout (per core):
  xs   [128, 32, 2048] bf16  x[b].T tiled d=(dt*128+p), streamed in 32 chunks
  wt   [128, 32, 128]  bf16  W o-tile (streamed, 3 bufs)
  arw  [128, 32, 128]  bf16  stationary cols: 0-63 lora_A (er), 64-127
                             router_W replicated 8x over r
  bta  [65, 2048]      bf16  rows 0-63: lora_B[half] as [er, o]; row 64: b_base
  haug [65, 2048]      bf16  router-scaled h; row 64 = ones (bias)
  out  [2048, 2048]    f32   result transposed: [o, t]

The h matmul (stationary=arw) yields h rows 0:63 AND router logit
partials rows 64:127 in one pass; logits = DVE row-sum over t.  Softmax
over the 64 replicated logits gives w/8 per row; scaling by 16 folds in
SCALING=2.  Each o-tile accumulates 32 d-matmuls + one K=65 lora/bias
matmul (rows of bta, ones-row of haug) into the same PSUM group.
"""
import sys

sys.path.insert(0, "/opt/trn_rl_repo")

import numpy as np
import ml_dtypes

import concourse.bass as bass
import concourse.mybir as mybir
import concourse.tile as tile
from concourse import bacc, bass_isa
from concourse.bass_utils import run_bass_kernel_spmd

F32 = mybir.dt.float32
BF16 = mybir.dt.bfloat16
NPBF = ml_dtypes.bfloat16

D, T, O_SH, E, R = 4096, 2048, 2048, 8, 8
ER = E * R  # 64
DT = D // 128  # 32 d-tiles
OT = O_SH // 128  # 16 o-tiles
NT4 = T // 512  # 4 psum-width chunks
ROUTER_TEMP = 1.0
SCALING = 16.0 / 8.0

_nc_cache = []


def build():
    nc = bacc.Bacc(None, target_bir_lowering=False)
    XS = nc.dram_tensor("XS", [128, DT * T], BF16, kind="ExternalInput")
    WT = nc.dram_tensor("WT", [OT * 128, DT * 128], BF16, kind="ExternalInput")
    ARW = nc.dram_tensor("ARW", [128, DT * 128], BF16, kind="ExternalInput")
    BTA = nc.dram_tensor("BTA", [ER + 1, O_SH], BF16, kind="ExternalInput")
    RB = nc.dram_tensor("RB", [ER], F32, kind="ExternalInput")
    out = nc.dram_tensor("out", [O_SH, T], F32, kind="ExternalOutput")

    with tile.TileContext(nc) as tc:
        with (
            tc.tile_pool(name="xp", bufs=1) as xp,
            tc.tile_pool(name="wp", bufs=3) as wp,
            tc.tile_pool(name="sg", bufs=1) as sg,
            tc.tile_pool(name="ev", bufs=2) as evp,
            tc.tile_pool(name="ps", bufs=8, space="PSUM") as psp,
        ):
            arw = sg.tile([128, DT, 128], BF16)
            nc.sync.dma_start(arw[:], ARW[:].rearrange("p (dt c) -> p dt c", c=128))
            bta = sg.tile([ER + 1, O_SH], BF16)
            nc.sync.dma_start(bta[:], BTA[:])
            rb = sg.tile([ER, 1], F32)
            nc.sync.dma_start(rb[:], RB[:, None])
            haug = sg.tile([ER + 1, T], BF16)
            nc.vector.memset(haug[ER : ER + 1, :], 1.0)

            xs = xp.tile([128, DT, T], BF16)
            for d in range(DT):
                nc.sync.dma_start(xs[:, d, :], XS[:, d * T : (d + 1) * T])

            def load_w(o):
                wt = wp.tile([128, DT, 128], BF16, tag="wt", name=f"wt{o}")
                nc.scalar.dma_start(
                    wt[:],
                    WT[o * 128 : (o + 1) * 128, :].rearrange(
                        "p (dt c) -> p dt c", c=128
                    ),
                )
                return wt

            # h (+ router logit partials) and o-tile 0, paced by the x stream
            wt0 = load_w(0)
            hps = [psp.tile([128, 512], F32, tag="ps", name=f"h{t}") for t in range(NT4)]
            ps0 = [psp.tile([128, 512], F32, tag="ps", name=f"p0_{t}") for t in range(NT4)]
            for d in range(DT):
                for t4 in range(NT4):
                    nc.tensor.matmul(
                        hps[t4][:],
                        arw[:, d, :],
                        xs[:, d, t4 * 512 : (t4 + 1) * 512],
                        start=(d == 0),
                        stop=(d == DT - 1),
                    )
                for t4 in range(NT4):
                    nc.tensor.matmul(
                        ps0[t4][:],
                        wt0[:, d, :],
                        xs[:, d, t4 * 512 : (t4 + 1) * 512],
                        start=(d == 0),
                        stop=False,
                    )

            # router: logits = rowsum_t(hps[64:128]) / T + rb; softmax over the
            # 64 replicated rows (sum = 8*S) -> w/8; *16 folds SCALING=2.
            lgq = sg.tile([ER, NT4], F32)
            for t4 in range(NT4):
                nc.vector.reduce_sum(
                    lgq[:, t4 : t4 + 1], hps[t4][ER:128, :], axis=mybir.AxisListType.X
                )
            lg = sg.tile([ER, 1], F32)
            nc.vector.reduce_sum(lg[:], lgq[:], axis=mybir.AxisListType.X)
            nc.scalar.activation(
                lg[:], lg[:], mybir.ActivationFunctionType.Copy,
                scale=1.0 / (T * ROUTER_TEMP),
            )
            nc.vector.tensor_tensor(lg[:], lg[:], rb[:], mybir.AluOpType.add)
            mx = sg.tile([ER, 1], F32)
            nc.gpsimd.partition_all_reduce(
                mx[:], lg[:], channels=ER, reduce_op=bass_isa.ReduceOp.max
            )
            nc.vector.tensor_tensor(lg[:], lg[:], mx[:], mybir.AluOpType.subtract)
            nc.scalar.activation(lg[:], lg[:], mybir.ActivationFunctionType.Exp)
            sm = sg.tile([ER, 1], F32)
            nc.gpsimd.partition_all_reduce(
                sm[:], lg[:], channels=ER, reduce_op=bass_isa.ReduceOp.add
            )
            rcp = sg.tile([ER, 1], F32)
            nc.vector.reciprocal(rcp[:], sm[:])
            w64 = sg.tile([ER, 1], F32)
            nc.vector.tensor_tensor(w64[:], lg[:], rcp[:], mybir.AluOpType.mult)
            nc.vector.tensor_scalar_mul(w64[:], w64[:], 8.0 * SCALING)
            for t4 in range(NT4):
                nc.vector.tensor_tensor(
                    haug[0:ER, t4 * 512 : (t4 + 1) * 512],
                    hps[t4][0:ER, :],
                    w64[:].to_broadcast([ER, 512]),
                    mybir.AluOpType.mult,
                )

            def lora_and_evac(o, pso):
                for t4 in range(NT4):
                    nc.tensor.matmul(
                        pso[t4][:],
                        bta[:, o * 128 : (o + 1) * 128],
                        haug[:, t4 * 512 : (t4 + 1) * 512],
                        start=False,
                        stop=True,
                    )
                ev = evp.tile([128, T], F32, tag="ev")
                for t4 in range(NT4):
                    nc.vector.tensor_copy(ev[:, t4 * 512 : (t4 + 1) * 512], pso[t4][:])
                nc.sync.dma_start(out[o * 128 : (o + 1) * 128, :], ev[:])

            prev = (0, ps0)
            for o in range(1, OT):
                wt = load_w(o)
                pso = [
                    psp.tile([128, 512], F32, tag="ps", name=f"p{o}_{t}")
                    for t in range(NT4)
                ]
                for d in range(DT):
                    for t4 in range(NT4):
                        nc.tensor.matmul(
                            pso[t4][:],
                            wt[:, d, :],
                            xs[:, d, t4 * 512 : (t4 + 1) * 512],
                            start=(d == 0),
                            stop=False,
                        )
                lora_and_evac(*prev)
                prev = (o, pso)
            lora_and_evac(*prev)
    nc.compile()
    return nc


def _get_nc():
    if not _nc_cache:
        _nc_cache.append(build())
    return _nc_cache[0]


def _tile_dmajor(a_dT):
    """[D, C] (d-major rows) -> [128, DT*C] with d = dt*128 + p."""
    Dd, C = a_dT.shape
    return np.ascontiguousarray(
        a_dT.reshape(Dd // 128, 128, C).swapaxes(0, 1)
    ).reshape(128, (Dd // 128) * C)


def kernel(x, W_base, b_base, lora_A, lora_B, router_W, router_b):
    x = np.asarray(x, dtype=np.float32)
    W_base = np.asarray(W_base, dtype=np.float32)
    b_base = np.asarray(b_base, dtype=np.float32)
    lora_A = np.asarray(lora_A, dtype=np.float32)
    lora_B = np.asarray(lora_B, dtype=np.float32)
    router_W = np.asarray(router_W, dtype=np.float32)
    router_b = np.asarray(router_b, dtype=np.float32)

    B, S, D_ = x.shape
    O = W_base.shape[0]

    xs_list = []
    for b in range(B):
        xt = np.ascontiguousarray(x[b].astype(NPBF).T)  # [D, T]
        xs_list.append(_tile_dmajor(xt))

    wt_list = []
    for hh in range(2):
        Wh = np.ascontiguousarray(W_base[hh * O_SH : (hh + 1) * O_SH].astype(NPBF).T)
        # [D, O_SH] -> [OT, 128, DT*128]
        w4 = Wh.reshape(DT, 128, OT, 128).transpose(2, 1, 0, 3)
        wt_list.append(np.ascontiguousarray(w4).reshape(OT * 128, DT * 128))

    At = lora_A.reshape(ER, D_)  # [er, d]
    rw64 = np.repeat(router_W, R, axis=0)  # [er, d]
    arw_h = _tile_dmajor(
        np.ascontiguousarray(np.concatenate([At, rw64], axis=0).astype(NPBF).T)
    )

    bta_list = []
    for hh in range(2):
        osl = slice(hh * O_SH, (hh + 1) * O_SH)
        Bt = lora_B[:, osl, :].transpose(0, 2, 1).reshape(ER, O_SH)
        bta_list.append(
            np.ascontiguousarray(
                np.concatenate([Bt, b_base[osl][None, :]], axis=0).astype(NPBF)
            )
        )
    rb64 = np.ascontiguousarray(np.repeat(router_b, R).astype(np.float32))

    in_maps = []
    for c in range(8):
        b, hh = c // 2, c % 2
        in_maps.append(
            {
                "XS": xs_list[b],
                "WT": wt_list[hh],
                "ARW": arw_h,
                "BTA": bta_list[hh],
                "RB": rb64,
            }
        )

    global _last_in_maps
    _last_in_maps = in_maps
    nc = _get_nc()
    res = run_bass_kernel_spmd(nc, in_maps, core_ids=list(range(8)))
    out = np.empty((B, S, O), dtype=np.float32)
    for c in range(8):
        b, hh = c // 2, c % 2
        out[b, :, hh * O_SH : (hh + 1) * O_SH] = res.results[c]["out"].T
    return out


# revision 5
# speedup vs baseline: 1.3822x; 1.2032x over previous
"""MoE-LoRA Linear kernel for 8 Trainium2 NeuronCores.

Sharding: core c -> (batch b = c//2, out-feature half = c%2).
Each core computes out[b, :, half] = x[b] @ W_half.T + b_half
                                   + SCALING * router-weighted LoRA.

All matmul operands are bf16 (fp32 accumulation in PSUM); x is fully
resident in SBUF so W is streamed exactly once.

Device layout (per core):
  xs   [128, 32, 2048] bf16  x[b].T tiled d=(dt*128+p), streamed in 32 chunks
  wt   [128, 32, 128]  bf16  W o-tile (streamed, 3 bufs)
  arw  [128, 32, 128]  bf16  stationary cols: 0-63 lora_A (er), 64-127
                             router_W replicated 8x over r
  bta  [65, 2048]      bf16  rows 0-63: lora_B[half] as [er, o]; row 64: b_base
  haug [65, 2048]      bf16  router-scaled h; row 64 = ones (bias)
  out  [2048, 2048]    f32   result transposed: [o, t]

The h matmul (stationary=arw) yields h rows 0:63 AND router logit
partials rows 64:127 in one pass; logits = DVE row-sum over t.  Softmax
over the 64 replicated logits gives w/8 per row; scaling by 16 folds in
SCALING=2.  Each o-tile accumulates 32 d-matmuls + one K=65 lora/bias
matmul (rows of bta, ones-row of haug) into the same PSUM group.
"""
import sys

sys.path.insert(0, "/opt/trn_rl_repo")

import numpy as np
import ml_dtypes

import concourse.bass as bass
import concourse.mybir as mybir
import concourse.tile as tile
from concourse import bacc, bass_isa
from concourse.bass_utils import run_bass_kernel_spmd

F32 = mybir.dt.float32
BF16 = mybir.dt.bfloat16
NPBF = ml_dtypes.bfloat16

D, T, O_SH, E, R = 4096, 2048, 2048, 8, 8
ER = E * R  # 64
DT = D // 128  # 32 d-tiles
OT = O_SH // 128  # 16 o-tiles
NT4 = T // 512  # 4 psum-width chunks
ROUTER_TEMP = 1.0
SCALING = 16.0 / 8.0

_nc_cache = []


def build():
    nc = bacc.Bacc(None, target_bir_lowering=False)
    XS = nc.dram_tensor("XS", [128, DT * T], BF16, kind="ExternalInput")
    WT = nc.dram_tensor("WT", [OT * 128, DT * 128], BF16, kind="ExternalInput")
    ARW = nc.dram_tensor("ARW", [128, DT * 128], BF16, kind="ExternalInput")
    BTA = nc.dram_tensor("BTA", [ER + 1, O_SH], BF16, kind="ExternalInput")
    RB = nc.dram_tensor("RB", [ER], F32, kind="ExternalInput")
    out = nc.dram_tensor("out", [O_SH, T], F32, kind="ExternalOutput")

    with tile.TileContext(nc) as tc:
        with (
            tc.tile_pool(name="xp", bufs=1) as xp,
            tc.tile_pool(name="wp", bufs=3) as wp,
            tc.tile_pool(name="sg", bufs=1) as sg,
            tc.tile_pool(name="ev", bufs=6) as evp,
            tc.tile_pool(name="ps", bufs=8, space="PSUM") as psp,
        ):
            arw = sg.tile([128, DT, 128], BF16)
            nc.sync.dma_start(arw[:], ARW[:].rearrange("p (dt c) -> p dt c", c=128))
            bta = sg.tile([ER + 1, O_SH], BF16)
            nc.sync.dma_start(bta[:], BTA[:])
            rb = sg.tile([ER, 1], F32)
            nc.sync.dma_start(rb[:], RB[:, None])
            haug = sg.tile([ER + 1, T], BF16)
            nc.vector.memset(haug[ER : ER + 1, :], 1.0)

            # x chunks go on the scalar HWDGE ring, issued up-front with no
            # slot waits, so nothing can starve the PE's x stream.  W/out
            # traffic shares the sync ring.
            xs = xp.tile([128, DT, T], BF16)
            for d in range(DT):
                nc.scalar.dma_start(xs[:, d, :], XS[:, d * T : (d + 1) * T])

            def load_w(o):
                wt = wp.tile([128, DT, 128], BF16, tag="wt", name=f"wt{o}")
                nc.sync.dma_start(
                    wt[:],
                    WT[o * 128 : (o + 1) * 128, :].rearrange(
                        "p (dt c) -> p dt c", c=128
                    ),
                )
                return wt

            # h (+ router logit partials) and o-tile 0, paced by the x stream
            wt0 = load_w(0)
            hps = [psp.tile([128, 512], F32, tag="ps", name=f"h{t}") for t in range(NT4)]
            ps0 = [psp.tile([128, 512], F32, tag="ps", name=f"p0_{t}") for t in range(NT4)]
            for d in range(DT):
                for t4 in range(NT4):
                    nc.tensor.matmul(
                        hps[t4][:],
                        arw[:, d, :],
                        xs[:, d, t4 * 512 : (t4 + 1) * 512],
                        start=(d == 0),
                        stop=(d == DT - 1),
                    )
                for t4 in range(NT4):
                    nc.tensor.matmul(
                        ps0[t4][:],
                        wt0[:, d, :],
                        xs[:, d, t4 * 512 : (t4 + 1) * 512],
                        start=(d == 0),
                        stop=False,
                    )

            # router: logits = rowsum_t(hps[64:128]) / T + rb; softmax over the
            # 64 replicated rows (sum = 8*S) -> w/8; *16 folds SCALING=2.
            lgq = sg.tile([ER, NT4], F32)
            for t4 in range(NT4):
                nc.vector.reduce_sum(
                    lgq[:, t4 : t4 + 1], hps[t4][ER:128, :], axis=mybir.AxisListType.X
                )
            lg = sg.tile([ER, 1], F32)
            nc.vector.reduce_sum(lg[:], lgq[:], axis=mybir.AxisListType.X)
            nc.scalar.activation(
                lg[:], lg[:], mybir.ActivationFunctionType.Copy,
                scale=1.0 / (T * ROUTER_TEMP),
            )
            nc.vector.tensor_tensor(lg[:], lg[:], rb[:], mybir.AluOpType.add)
            mx = sg.tile([ER, 1], F32)
            nc.gpsimd.partition_all_reduce(
                mx[:], lg[:], channels=ER, reduce_op=bass_isa.ReduceOp.max
            )
            nc.vector.tensor_tensor(lg[:], lg[:], mx[:], mybir.AluOpType.subtract)
            nc.scalar.activation(lg[:], lg[:], mybir.ActivationFunctionType.Exp)
            sm = sg.tile([ER, 1], F32)
            nc.gpsimd.partition_all_reduce(
                sm[:], lg[:], channels=ER, reduce_op=bass_isa.ReduceOp.add
            )
            rcp = sg.tile([ER, 1], F32)
            nc.vector.reciprocal(rcp[:], sm[:])
            w64 = sg.tile([ER, 1], F32)
            nc.vector.tensor_tensor(w64[:], lg[:], rcp[:], mybir.AluOpType.mult)
            nc.vector.tensor_scalar_mul(w64[:], w64[:], 8.0 * SCALING)
            for t4 in range(NT4):
                nc.vector.tensor_tensor(
                    haug[0:ER, t4 * 512 : (t4 + 1) * 512],
                    hps[t4][0:ER, :],
                    w64[:].to_broadcast([ER, 512]),
                    mybir.AluOpType.mult,
                )

            def lora_and_evac(o, pso):
                for t4 in range(NT4):
                    nc.tensor.matmul(
                        pso[t4][:],
                        bta[:, o * 128 : (o + 1) * 128],
                        haug[:, t4 * 512 : (t4 + 1) * 512],
                        start=False,
                        stop=True,
                    )
                for t4 in range(NT4):
                    ev = evp.tile([128, 512], F32, tag="ev")
                    nc.vector.tensor_copy(ev[:], pso[t4][:])
                    nc.sync.dma_start(
                        out[o * 128 : (o + 1) * 128, t4 * 512 : (t4 + 1) * 512], ev[:]
                    )

            prev = (0, ps0)
            for o in range(1, OT):
                wt = load_w(o)
                pso = [
                    psp.tile([128, 512], F32, tag="ps", name=f"p{o}_{t}")
                    for t in range(NT4)
                ]
                for d in range(DT):
                    for t4 in range(NT4):
                        nc.tensor.matmul(
                            pso[t4][:],
                            wt[:, d, :],
                            xs[:, d, t4 * 512 : (t4 + 1) * 512],
                            start=(d == 0),
                            stop=False,
                        )
                lora_and_evac(*prev)
                prev = (o, pso)
            lora_and_evac(*prev)
    nc.compile()
    return nc


def _get_nc():
    if not _nc_cache:
        _nc_cache.append(build())
    return _nc_cache[0]


def _tile_dmajor(a_dT):
    """[D, C] (d-major rows) -> [128, DT*C] with d = dt*128 + p."""
    Dd, C = a_dT.shape
    return np.ascontiguousarray(
        a_dT.reshape(Dd // 128, 128, C).swapaxes(0, 1)
    ).reshape(128, (Dd // 128) * C)


def kernel(x, W_base, b_base, lora_A, lora_B, router_W, router_b):
    x = np.asarray(x, dtype=np.float32)
    W_base = np.asarray(W_base, dtype=np.float32)
    b_base = np.asarray(b_base, dtype=np.float32)
    lora_A = np.asarray(lora_A, dtype=np.float32)
    lora_B = np.asarray(lora_B, dtype=np.float32)
    router_W = np.asarray(router_W, dtype=np.float32)
    router_b = np.asarray(router_b, dtype=np.float32)

    B, S, D_ = x.shape
    O = W_base.shape[0]

    xs_list = []
    for b in range(B):
        xt = np.ascontiguousarray(x[b].astype(NPBF).T)  # [D, T]
        xs_list.append(_tile_dmajor(xt))

    wt_list = []
    for hh in range(2):
        Wh = np.ascontiguousarray(W_base[hh * O_SH : (hh + 1) * O_SH].astype(NPBF).T)
        # [D, O_SH] -> [OT, 128, DT*128]
        w4 = Wh.reshape(DT, 128, OT, 128).transpose(2, 1, 0, 3)
        wt_list.append(np.ascontiguousarray(w4).reshape(OT * 128, DT * 128))

    At = lora_A.reshape(ER, D_)  # [er, d]
    rw64 = np.repeat(router_W, R, axis=0)  # [er, d]
    arw_h = _tile_dmajor(
        np.ascontiguousarray(np.concatenate([At, rw64], axis=0).astype(NPBF).T)
    )

    bta_list = []
    for hh in range(2):
        osl = slice(hh * O_SH, (hh + 1) * O_SH)
        Bt = lora_B[:, osl, :].transpose(0, 2, 1).reshape(ER, O_SH)
        bta_list.append(
            np.ascontiguousarray(
                np.concatenate([Bt, b_base[osl][None, :]], axis=0).astype(NPBF)
            )
        )
    rb64 = np.ascontiguousarray(np.repeat(router_b, R).astype(np.float32))

    in_maps = []
    for c in range(8):
        b, hh = c // 2, c % 2
        in_maps.append(
            {
                "XS": xs_list[b],
                "WT": wt_list[hh],
                "ARW": arw_h,
                "BTA": bta_list[hh],
                "RB": rb64,
            }
        )

    global _last_in_maps
    _last_in_maps = in_maps
    nc = _get_nc()
    res = run_bass_kernel_spmd(nc, in_maps, core_ids=list(range(8)))
    out = np.empty((B, S, O), dtype=np.float32)
    for c in range(8):
        b, hh = c // 2, c % 2
        out[b, :, hh * O_SH : (hh + 1) * O_SH] = res.results[c]["out"].T
    return out
